# revision 3
# baseline (speedup 1.0000x reference)
"""2-layer GraphSAGE (mean agg) on 8 TRN2 NeuronCores via Bass/Tile.

Sharding: degree-sort nodes, deal round-robin over 8 cores so every core's
128-node block b has the same padded slot count G_b -> one SPMD program.
Per core: prologue computes x2 = [x@W1_l | x@W1_r + b1] for its shard
(matmul with hstacked weights + PE transposes); AllGather of the x@W1_l half
gives the layer-1 gather table. Layer 1: per edge-slot indirect-DMA gather of
128 rows + identity-matmul PSUM accumulation (= segment mean after invdeg
scale), fused epilogue on DVE, inline transform to h2 = [h@W2_l | h@W2_r+b2];
AllGather of h@W2_l half; layer 2 repeats the gather-accumulate -> output.
Self-halves never leave SBUF. Padding slots point at a guaranteed-zero row.
"""
import sys

for p in ("/opt/trn_rl_repo", "/root/.axon_site/_ro/trn_rl_repo"):
    if p not in sys.path:
        sys.path.insert(0, p)

import numpy as np
import ml_dtypes

import concourse.bacc as bacc
import concourse.mybir as mybir
import concourse.tile as tile
from concourse.bass import IndirectOffsetOnAxis
from concourse.bass_utils import run_bass_kernel_spmd
from concourse.masks import make_identity

P = 128
NCORES = 8
N = 100000
CIN, CHID, COUT = 64, 64, 32
NC_REAL = N // NCORES            # 12500
NB = (NC_REAL + P - 1) // P      # 98
NC_PAD = NB * P                  # 12544
N_ALL = NCORES * NC_PAD          # 100352
ZPOS = NC_REAL                   # core0 dead row -> global zero row
SLOTCAP_G = 256                  # max sum(G_b) per idx-tile batch

bf16 = mybir.dt.bfloat16
f32 = mybir.dt.float32
i32 = mybir.dt.int32


def _build_plan(src, tgt):
    deg = np.bincount(tgt, minlength=N).astype(np.int64)
    order = np.argsort(deg, kind="stable")
    pos = np.empty(N, np.int64)
    pos[order] = (np.arange(N) % NCORES) * NC_PAD + (np.arange(N) // NCORES)
    ds = np.zeros(NB * P * NCORES, np.int64)
    ds[:N] = deg[order]
    G = np.maximum(ds.reshape(NB, P * NCORES).max(axis=1), 1).astype(np.int64)
    sbs, cur, acc = [], [], 0
    for b in range(NB):
        if cur and acc + G[b] > SLOTCAP_G:
            sbs.append(cur); cur, acc = [], 0
        cur.append(b); acc += G[b]
    if cur:
        sbs.append(cur)
    e_pos_t = pos[tgt]
    e_core = e_pos_t // NC_PAD
    e_j = e_pos_t % NC_PAD
    e_src = pos[src].astype(np.int32)
    o = np.lexsort((e_j, e_core))
    e_core, e_j, e_src = e_core[o], e_j[o], e_src[o]
    col_off = np.zeros(NB, np.int64)
    sb_base = np.zeros(NB, np.int64)
    Gsb_of_b = np.zeros(NB, np.int64)
    base = 0
    for sb in sbs:
        off = 0
        for b in sb:
            col_off[b] = off; sb_base[b] = base; off += G[b]
        for b in sb:
            Gsb_of_b[b] = off
        base += P * off
    Gtot = int(G.sum())
    idx_flat = np.full((NCORES, P * Gtot), ZPOS, np.int32)
    for k in range(NCORES):
        m = e_core == k
        j, sp = e_j[m], e_src[m]
        grp_start = np.searchsorted(j, np.arange(NC_PAD), side="left")
        slot = np.arange(j.size) - grp_start[j]
        b, pp = j // P, j % P
        idx_flat[k, sb_base[b] + pp * Gsb_of_b[b] + col_off[b] + slot] = sp
    invdeg = np.zeros(N, np.float32)
    invdeg[deg > 0] = 1.0 / deg[deg > 0]
    invdeg_pc = np.zeros((NCORES, P, NB), np.float32)
    nodes_per_core = []
    for k in range(NCORES):
        nodes_k = order[np.arange(NC_REAL) * NCORES + k]
        nodes_per_core.append(nodes_k)
        ivp = np.zeros(NC_PAD, np.float32)
        ivp[:NC_REAL] = invdeg[nodes_k]
        invdeg_pc[k] = ivp.reshape(NB, P).T
    sb_bases = {sb[0]: int(sb_base[sb[0]]) for sb in sbs}
    return dict(G=G, sbs=sbs, idx_flat=idx_flat, invdeg_pc=invdeg_pc,
                nodes_per_core=nodes_per_core, Gtot=Gtot, sb_bases=sb_bases)


def _build_nc(G, sbs, Gtot, sb_bases):
    nc = bacc.Bacc("TRN2", target_bir_lowering=False, debug=False,
                   num_devices=NCORES)
    xT_d = nc.dram_tensor("xT", [CIN, NC_PAD], bf16, kind="ExternalInput")
    idx_d = nc.dram_tensor("idx", [P * Gtot], i32, kind="ExternalInput")
    inv_d = nc.dram_tensor("invdeg", [P, NB], f32, kind="ExternalInput")
    w1_d = nc.dram_tensor("W1comb", [CIN, 2 * CHID], bf16, kind="ExternalInput")
    w2_d = nc.dram_tensor("W2comb", [CHID, 2 * COUT], bf16, kind="ExternalInput")
    b1_d = nc.dram_tensor("b1c", [2 * CHID, 1], f32, kind="ExternalInput")
    b2_d = nc.dram_tensor("b2c", [2 * COUT, 1], f32, kind="ExternalInput")
    out_d = nc.dram_tensor("out", [NC_PAD, COUT], f32, kind="ExternalOutput")

    with tile.TileContext(nc) as tc:
        with (
            tc.tile_pool(name="consts", bufs=1) as consts,
            tc.tile_pool(name="x2keep", bufs=NB) as x2keep,
            tc.tile_pool(name="h2keep", bufs=NB) as h2keep,
            tc.tile_pool(name="io", bufs=3) as io,
            tc.tile_pool(name="gat", bufs=3) as gat,
            tc.tile_pool(name="msgp", bufs=8) as msgp,
            tc.tile_pool(name="blk", bufs=3) as blk,
            tc.tile_pool(name="ps", bufs=1, space="PSUM") as ps,
            tc.tile_pool(name="dram", bufs=1, space="DRAM") as dram,
        ):
            ident = consts.tile([P, P], bf16)
            make_identity(nc, ident[:])
            w1_s = consts.tile([CIN, 2 * CHID], bf16)
            nc.sync.dma_start(out=w1_s[:], in_=w1_d[:])
            w2_s = consts.tile([CHID, 2 * COUT], bf16)
            nc.sync.dma_start(out=w2_s[:], in_=w2_d[:])
            b1_s = consts.tile([2 * CHID, 1], f32)
            nc.sync.dma_start(out=b1_s[:], in_=b1_d[:])
            b2_s = consts.tile([2 * COUT, 1], f32)
            nc.sync.dma_start(out=b2_s[:], in_=b2_d[:])
            inv_s = consts.tile([P, NB], f32)
            nc.sync.dma_start(out=inv_s[:], in_=inv_d[:])

            x2l_shard = dram.tile([NC_PAD, CHID], bf16)
            x2l_full = dram.tile([N_ALL, CHID], bf16, addr_space="Shared")
            h2l_shard = dram.tile([NC_PAD, COUT], bf16)
            h2l_full = dram.tile([N_ALL, COUT], bf16, addr_space="Shared")

            # ---- prologue: x2 = [x@W1_l | x@W1_r + b1] ----
            x2_tiles = []
            for b in range(NB):
                xT_t = io.tile([CIN, P], bf16, tag="xTt")
                nc.sync.dma_start(out=xT_t[:], in_=xT_d[:, b * P:(b + 1) * P])
                ps1 = ps.tile([2 * CHID, P], f32, tag="pro1")
                nc.tensor.matmul(ps1[:], lhsT=w1_s[:], rhs=xT_t[:],
                                 start=True, stop=True)
                x2T_t = blk.tile([2 * CHID, P], bf16, tag="x2T")
                nc.scalar.activation(x2T_t[:], ps1[:],
                                     mybir.ActivationFunctionType.Identity,
                                     bias=b1_s[:, :1], scale=1.0)
                ps2 = ps.tile([P, 2 * CHID], bf16, tag="pro2")
                nc.tensor.transpose(ps2[:], x2T_t[:], ident[:])
                x2_s = x2keep.tile([P, 2 * CHID], bf16, tag="x2s")
                nc.vector.tensor_copy(out=x2_s[:], in_=ps2[:])
                nc.sync.dma_start(out=x2l_shard[b * P:(b + 1) * P, :],
                                  in_=x2_s[:, :CHID])
                x2_tiles.append(x2_s)
            zt = consts.tile([P, CHID], bf16)
            nc.vector.memset(zt[:], 0.0)
            nc.sync.dma_start(out=x2l_shard[NC_REAL:NC_PAD, :],
                              in_=zt[:NC_PAD - NC_REAL, :])
            nc.gpsimd.collective_compute(
                "AllGather", mybir.AluOpType.bypass,
                replica_groups=[list(range(NCORES))],
                ins=[x2l_shard.opt()], outs=[x2l_full.opt()])

            # ---- layer 1 + inline h->h2 ----
            h2_tiles = []
            for sb in sbs:
                gsb = int(sum(int(G[b]) for b in sb))
                base = sb_bases[sb[0]]
                idx_t = gat.tile([P, gsb], i32, tag="idx")
                nc.sync.dma_start(
                    out=idx_t[:],
                    in_=idx_d[base:base + P * gsb].rearrange("(p g) -> p g", p=P))
                off = 0
                for b in sb:
                    gb = int(G[b])
                    agg = ps.tile([P, CHID], f32, tag="agg", bufs=2)
                    for g in range(gb):
                        msg = msgp.tile([P, CHID], bf16, tag="msg")
                        nc.gpsimd.indirect_dma_start(
                            out=msg[:], out_offset=None, in_=x2l_full[:],
                            in_offset=IndirectOffsetOnAxis(
                                ap=idx_t[:, off + g:off + g + 1], axis=0))
                        nc.tensor.matmul(agg[:], lhsT=ident[:], rhs=msg[:],
                                         start=(g == 0), stop=(g == gb - 1))
                    off += gb
                    tmp = blk.tile([P, CHID], f32, tag="tmp1")
                    nc.vector.scalar_tensor_tensor(
                        out=tmp[:], in0=agg[:], scalar=inv_s[:, b:b + 1],
                        in1=x2_tiles[b][:, CHID:2 * CHID],
                        op0=mybir.AluOpType.mult, op1=mybir.AluOpType.add)
                    h_t = blk.tile([P, CHID], bf16, tag="ht")
                    nc.vector.scalar_tensor_tensor(
                        out=h_t[:], in0=tmp[:], scalar=0.01, in1=tmp[:],
                        op0=mybir.AluOpType.mult, op1=mybir.AluOpType.max)
                    psT = ps.tile([CHID, P], bf16, tag="psT")
                    nc.tensor.transpose(psT[:], h_t[:], ident[:])
                    hT_t = blk.tile([CHID, P], bf16, tag="hTt")
                    nc.scalar.copy(out=hT_t[:], in_=psT[:])
                    ps3 = ps.tile([2 * COUT, P], f32, tag="ps3")
                    nc.tensor.matmul(ps3[:], lhsT=w2_s[:], rhs=hT_t[:],
                                     start=True, stop=True)
                    h2T_t = blk.tile([2 * COUT, P], bf16, tag="h2Tt")
                    nc.scalar.activation(h2T_t[:], ps3[:],
                                         mybir.ActivationFunctionType.Identity,
                                         bias=b2_s[:, :1], scale=1.0)
                    ps4 = ps.tile([P, 2 * COUT], bf16, tag="ps4")
                    nc.tensor.transpose(ps4[:], h2T_t[:],
                                        ident[:2 * COUT, :2 * COUT])
                    h2_s = h2keep.tile([P, 2 * COUT], bf16, tag="h2s")
                    nc.vector.tensor_copy(out=h2_s[:], in_=ps4[:])
                    nc.sync.dma_start(out=h2l_shard[b * P:(b + 1) * P, :],
                                      in_=h2_s[:, :COUT])
                    h2_tiles.append(h2_s)
            zt2 = consts.tile([P, COUT], bf16)
            nc.vector.memset(zt2[:], 0.0)
            nc.sync.dma_start(out=h2l_shard[NC_REAL:NC_PAD, :],
                              in_=zt2[:NC_PAD - NC_REAL, :])
            nc.gpsimd.collective_compute(
                "AllGather", mybir.AluOpType.bypass,
                replica_groups=[list(range(NCORES))],
                ins=[h2l_shard.opt()], outs=[h2l_full.opt()])

            # ---- layer 2 ----
            for sb in sbs:
                gsb = int(sum(int(G[b]) for b in sb))
                base = sb_bases[sb[0]]
                idx_t = gat.tile([P, gsb], i32, tag="idx")
                nc.sync.dma_start(
                    out=idx_t[:],
                    in_=idx_d[base:base + P * gsb].rearrange("(p g) -> p g", p=P))
                off = 0
                for b in sb:
                    gb = int(G[b])
                    agg = ps.tile([P, COUT], f32, tag="agg", bufs=2)
                    for g in range(gb):
                        msg = msgp.tile([P, COUT], bf16, tag="msg2")
                        nc.gpsimd.indirect_dma_start(
                            out=msg[:], out_offset=None, in_=h2l_full[:],
                            in_offset=IndirectOffsetOnAxis(
                                ap=idx_t[:, off + g:off + g + 1], axis=0))
                        nc.tensor.matmul(agg[:], lhsT=ident[:], rhs=msg[:],
                                         start=(g == 0), stop=(g == gb - 1))
                    off += gb
                    tmp = blk.tile([P, COUT], f32, tag="tmp2")
                    nc.vector.scalar_tensor_tensor(
                        out=tmp[:], in0=agg[:], scalar=inv_s[:, b:b + 1],
                        in1=h2_tiles[b][:, COUT:2 * COUT],
                        op0=mybir.AluOpType.mult, op1=mybir.AluOpType.add)
                    out_t = blk.tile([P, COUT], f32, tag="outt")
                    nc.vector.scalar_tensor_tensor(
                        out=out_t[:], in0=tmp[:], scalar=0.01, in1=tmp[:],
                        op0=mybir.AluOpType.mult, op1=mybir.AluOpType.max)
                    nc.sync.dma_start(out=out_d[b * P:(b + 1) * P, :],
                                      in_=out_t[:])
    nc.compile()
    return nc


def kernel(x, edge_index, W1_l, b1, W1_r, W2_l, b2, W2_r, _want_trace=False):
    x = np.asarray(x, np.float32)
    ei = np.asarray(edge_index).astype(np.int64)
    plan = _build_plan(ei[0], ei[1])
    nc = _build_nc(plan["G"], plan["sbs"], plan["Gtot"], plan["sb_bases"])
    W1c = np.hstack([np.asarray(W1_l, np.float32),
                     np.asarray(W1_r, np.float32)]).astype(ml_dtypes.bfloat16)
    W2c = np.hstack([np.asarray(W2_l, np.float32),
                     np.asarray(W2_r, np.float32)]).astype(ml_dtypes.bfloat16)
    b1c = np.concatenate([np.zeros(CHID, np.float32),
                          np.asarray(b1, np.float32)])[:, None]
    b2c = np.concatenate([np.zeros(COUT, np.float32),
                          np.asarray(b2, np.float32)])[:, None]
    in_maps = []
    for k in range(NCORES):
        nodes_k = plan["nodes_per_core"][k]
        xTs = np.zeros((CIN, NC_PAD), np.float32)
        xTs[:, :NC_REAL] = x[nodes_k].T
        in_maps.append({
            "xT": xTs.astype(ml_dtypes.bfloat16),
            "idx": plan["idx_flat"][k],
            "invdeg": plan["invdeg_pc"][k],
            "W1comb": W1c, "W2comb": W2c, "b1c": b1c, "b2c": b2c,
        })
    res = run_bass_kernel_spmd(nc, in_maps, list(range(NCORES)),
                               trace=_want_trace)
    out = np.zeros((N, COUT), np.float32)
    for k in range(NCORES):
        out[plan["nodes_per_core"][k]] = res.results[k]["out"][:NC_REAL]
    kernel._last_exec_ns = res.exec_time_ns
    return out
